# revision 2
# baseline (speedup 1.0000x reference)
"""GNN edge-gate kernel, v2.

Math: g[v] = relu(emb[v] @ W_emb + b_emb) @ (w1+w2)/2 + b_edge/2  (per-node scalar)
      out_e = sigmoid(log(eps)-log1p(-eps) + g[src_e] + g[dst_e]),
      eps = a1*noise + b1.

Layout trick: host buckets each core's 200K edges by w = src % 3136
(= column of the SBUF g-table, 16 partition-candidates per column,
c = src // 3136 selected by mask).  Main slots are a STATIC map
slot->bucket, so the src "gather" is an affine broadcast re-read of the
table (zero POOL work).  Only dst stays random: split between POOL
indirect_copy and big SWDGE dma_gathers of 256B bf16 g-rows
(row = v>>7, c128 = v&127 selected by mask).  Bucket overflow goes to
spill tiles (both endpoints random).  Host inverse-permutes the output.
"""
import sys
sys.path.insert(0, '/opt/trn_rl_repo')
import numpy as np

N, IN_DIM, HID = 50000, 256, 64
E = 1_600_000
BIAS = 0.0001
NCORES = 8
EC = E // NCORES

TAB_W = 3136              # buckets / table width; 16*3136 = 50176 >= N
GPAD = 16 * TAB_W         # padded g length
M = 512
TILE_E = 4096
NMAIN = TAB_W // 64       # 49 main tiles (64 buckets x 64 slots each)
NSPILL = 3
NT = NMAIN + NSPILL       # 52
ECP = NT * TILE_E         # 212992 slots per core
NNC = N // NCORES         # 6250
NNCP = 6272
RT_ROWS = GPAD // 128     # 392 bf16 rows of 128 (256B each)

N_SW = 0                  # main tiles routed via SWDGE dma_gather (0: POOL-only wins)
SPILL_DST_SW = False      # spill-tile dst endpoint via SWDGE (else POOL)
NI = 512                  # idxs per dma_gather (hard ucode limit)
NQ = 1                    # SWDGE queues (multi-queue completion tracking is racy)

if N_SW == 0:
    ROUTE_SW = [False] * NMAIN
else:
    _pool_main = sorted(set(min(NMAIN - 1,
                                int((i + 0.5) * NMAIN / max(NMAIN - N_SW, 1)))
                            for i in range(NMAIN - N_SW)))
    ROUTE_SW = [t not in _pool_main for t in range(NMAIN)]   # main tiles

_nc_cache = {}


def _super_of(t):
    if t < 48:
        return t // 16, t % 16
    return 3, t - 48


def _build(repeat=1):
    from concourse import bass, bacc, tile, mybir

    f32 = mybir.dt.float32
    bf16 = mybir.dt.bfloat16
    i16 = mybir.dt.int16
    u16 = mybir.dt.uint16
    ACT = mybir.ActivationFunctionType
    OP = mybir.AluOpType
    nc = bacc.Bacc("TRN2", target_bir_lowering=False, debug=False,
                   num_devices=NCORES, num_swdge_queues=NQ)

    embT = nc.dram_tensor("embT", [2, 128, NNCP], f32, kind="ExternalInput")
    idxAu_d = nc.dram_tensor("idxAu", [NT * TILE_E], u16, kind="ExternalInput")
    idxAi_d = nc.dram_tensor("idxAi", [NT * TILE_E], i16, kind="ExternalInput")
    cB_d = nc.dram_tensor("cB", [NT * 2 * TILE_E], f32, kind="ExternalInput")
    idxB_d = nc.dram_tensor("idxB", [NSPILL * TILE_E], u16, kind="ExternalInput")
    noise_d = nc.dram_tensor("noise", [ECP], f32, kind="ExternalInput")
    wemb_d = nc.dram_tensor("wemb", [2, 128, HID], f32, kind="ExternalInput")
    bemb_d = nc.dram_tensor("bemb", [HID, 1], f32, kind="ExternalInput")
    wbar_d = nc.dram_tensor("wbar", [HID, 1], f32, kind="ExternalInput")
    bhalf_d = nc.dram_tensor("bhalf", [1, 1], f32, kind="ExternalInput")
    expand8_d = nc.dram_tensor("expand8", [8, 128], f32, kind="ExternalInput")
    bdiag8_d = nc.dram_tensor("bdiag8", [128, 8], f32, kind="ExternalInput")
    iota16_d = nc.dram_tensor("iota16", [128, 1], f32, kind="ExternalInput")
    iota128_d = nc.dram_tensor("iota128", [128, 1], f32, kind="ExternalInput")
    sel8_d = nc.dram_tensor("sel8", [8, 8, 128], f32, kind="ExternalInput")
    oc8_d = nc.dram_tensor("oc8", [8, 128, 8], f32, kind="ExternalInput")
    out_d = nc.dram_tensor("out", [ECP], f32, kind="ExternalOutput")

    a1, b1 = 2.0 * BIAS - 1.0, 1.0 - BIAS
    a2, b2 = 1.0 - 2.0 * BIAS, BIAS

    with tile.TileContext(nc) as tc:
        with tc.tile_pool(name="const", bufs=1) as cp, \
             tc.tile_pool(name="tab", bufs=1) as tabp, \
             tc.tile_pool(name="dram", bufs=1, space="DRAM") as dram:
            def cload(name, shape, dt, src):
                t = cp.tile(shape, dt, tag=name)
                nc.sync.dma_start(out=t[:], in_=src)
                return t
            w0 = cload("w0", [128, HID], f32, wemb_d[0])
            w1 = cload("w1", [128, HID], f32, wemb_d[1])
            bemb = cload("bemb", [HID, 1], f32, bemb_d[:, :])
            wbar = cload("wbar", [HID, 1], f32, wbar_d[:, :])
            bhalf = cload("bhalf", [1, 1], f32, bhalf_d[:, :])
            expand8 = cload("ex8", [8, 128], f32, expand8_d[:, :])
            bdiag8 = cload("bd8", [128, 8], f32, bdiag8_d[:, :])
            iota16 = cload("io16", [128, 1], f32, iota16_d[:, :])
            iota128 = cload("io128", [128, 1], f32, iota128_d[:, :])
            sel8 = [cload(f"sel8_{k}", [8, 128], f32, sel8_d[k]) for k in range(8)]
            oc8 = [cload(f"oc8_{k}", [128, 8], f32, oc8_d[k]) for k in range(8)]
            a1t = cp.tile([128, 1], f32, tag="a1t"); nc.vector.memset(a1t[:], a1)
            b1t = cp.tile([128, 1], f32, tag="b1t"); nc.vector.memset(b1t[:], b1)
            a2t = cp.tile([128, 1], f32, tag="a2t"); nc.vector.memset(a2t[:], a2)
            b2t = cp.tile([128, 1], f32, tag="b2t"); nc.vector.memset(b2t[:], b2)
            zpad = cp.tile([1, GPAD - N], f32, tag="zpad")
            nc.vector.memset(zpad[:], 0.0)

            table = tabp.tile([128, TAB_W], f32, tag="table")
            g_all = dram.tile([1, GPAD], f32, tag="gall")
            g16_d = dram.tile([1, GPAD], bf16, tag="g16")

            for rep in range(repeat):
                # ============ phase 1: per-node scalar g ============
                g_sb = cp.tile([1, NNCP], f32, tag="gsb")
                with tc.tile_pool(name="p1", bufs=3) as p1, \
                     tc.tile_pool(name="ps1", bufs=2, space="PSUM") as ps1, \
                     tc.tile_pool(name="ps1g", bufs=2, space="PSUM") as ps1g:
                    col = 0
                    while col < NNCP:
                        n = min(512, NNCP - col)
                        r0 = p1.tile([128, n], f32, tag="r0")
                        r1 = p1.tile([128, n], f32, tag="r1")
                        nc.sync.dma_start(out=r0[:], in_=embT[0, :, col:col + n])
                        nc.sync.dma_start(out=r1[:], in_=embT[1, :, col:col + n])
                        ph = ps1.tile([HID, n], f32, tag="ph")
                        nc.tensor.matmul(out=ph[:], lhsT=w0[:], rhs=r0[:],
                                         start=True, stop=False)
                        nc.tensor.matmul(out=ph[:], lhsT=w1[:], rhs=r1[:],
                                         start=False, stop=True)
                        hT = p1.tile([HID, n], f32, tag="hT")
                        nc.scalar.activation(out=hT[:], in_=ph[:], func=ACT.Relu,
                                             bias=bemb[:, 0:1])
                        pg = ps1g.tile([1, n], f32, tag="pg")
                        nc.tensor.matmul(out=pg[:], lhsT=wbar[:], rhs=hT[:],
                                         start=True, stop=True)
                        nc.scalar.activation(out=g_sb[0:1, col:col + n], in_=pg[:],
                                             func=ACT.Identity,
                                             bias=bhalf[0:1, 0:1])
                        col += n

                g_mine = dram.tile([1, NNC], f32, tag="gmine")
                nc.sync.dma_start(out=g_mine[:], in_=g_sb[0:1, 0:NNC])
                nc.gpsimd.collective_compute(
                    "AllGather", bass.mybir.AluOpType.bypass,
                    replica_groups=[list(range(NCORES))],
                    ins=[g_mine[:].opt()], outs=[g_all[0:1, 0:N].opt()])
                nc.sync.dma_start(out=g_all[0:1, N:GPAD], in_=zpad[:])

                # POOL table: table[16G+c, w] = g[3136c + w]
                for G in range(8):
                    nc.sync.dma_start(
                        out=table[16 * G:16 * G + 16, :],
                        in_=g_all[0].rearrange("(c w) -> c w", w=TAB_W))
                # bf16 row table: 392 rows x 128 (256B rows)
                g16sb = cp.tile([128, RT_ROWS], f32, tag="g16sbf")
                nc.sync.dma_start(out=g16sb[:],
                                  in_=g_all[0].rearrange("(p f) -> p f", p=128))
                g16sbb = cp.tile([128, RT_ROWS], bf16, tag="g16sbb")
                nc.vector.tensor_copy(out=g16sbb[:], in_=g16sb[:])
                nc.sync.dma_start(out=g16_d[0].rearrange("(p f) -> p f", p=128),
                                  in_=g16sbb[:])
                tab16 = g16_d[0].rearrange("(r x) -> r x", x=128)

                # ============ phase 2: edges ============
                with tc.tile_pool(name="st", bufs=4) as stp, \
                     tc.tile_pool(name="idx", bufs=3) as idxp, \
                     tc.tile_pool(name="cnd", bufs=2) as cndp, \
                     tc.tile_pool(name="gth", bufs=3) as gthp, \
                     tc.tile_pool(name="msk", bufs=3) as mskp, \
                     tc.tile_pool(name="gate", bufs=1) as gatep, \
                     tc.tile_pool(name="acc", bufs=2) as accp, \
                     tc.tile_pool(name="pss", bufs=1, space="PSUM") as pssp, \
                     tc.tile_pool(name="psd", bufs=2, space="PSUM") as psdp, \
                     tc.tile_pool(name="psb", bufs=2, space="PSUM") as psbp:
                    acc = None
                    qrr = 0
                    for t in range(NT):
                        s, tt = _super_of(t)
                        P = 128 if s < 3 else 32
                        if tt == 0:
                            acc = accp.tile([P, M], f32, tag=f"acc{s % 2}")
                        ps_s = pssp.tile([8, M], f32, tag=f"ps{t % 2}")
                        spill = t >= NMAIN
                        sw = SPILL_DST_SW if spill else ROUTE_SW[t]
                        base = t * TILE_E
                        r0 = 8 * tt

                        # ---------- src ----------
                        c0 = stp.tile([8, M], f32, tag="c0")
                        nc.sync.dma_start(
                            out=c0[:],
                            in_=cB_d[2 * base:2 * base + TILE_E].rearrange(
                                "(g m) -> g m", g=8))
                        psd0 = psdp.tile([128, M], f32, tag="psd")
                        nc.tensor.matmul(out=psd0[:], lhsT=expand8[:], rhs=c0[:],
                                         start=True, stop=True)
                        msk0 = mskp.tile([128, M], f32, tag="msk0")
                        if not spill:
                            nc.vector.scalar_tensor_tensor(
                                out=msk0[:].rearrange("p (b r) -> p b r", r=8),
                                in0=psd0[:].rearrange("p (b r) -> p b r", r=8),
                                scalar=iota16[:, 0:1],
                                in1=table[:, 64 * t:64 * t + 64].broadcast_to(
                                    [128, 64, 8]),
                                op0=OP.is_equal, op1=OP.mult)
                        else:
                            ei0 = idxp.tile([128, M // 16], u16, tag="ei0")
                            nc.sync.dma_start(
                                out=ei0[:],
                                in_=idxAu_d[base:base + TILE_E].rearrange(
                                    "(p w) -> p w", p=128))
                            cand0 = cndp.tile([128, M], f32, tag="cand0")
                            nc.gpsimd.indirect_copy(
                                out=cand0[:], data=table[:], idxs=ei0[:],
                                i_know_ap_gather_is_preferred=True)
                            nc.vector.scalar_tensor_tensor(
                                out=msk0[:], in0=psd0[:], scalar=iota16[:, 0:1],
                                in1=cand0[:], op0=OP.is_equal, op1=OP.mult)
                        nc.tensor.matmul(out=ps_s[:], lhsT=bdiag8[:],
                                         rhs=msk0[:], start=True, stop=False,
                                         skip_group_check=True)

                        # ---------- dst ----------
                        if not sw:
                            ei1 = idxp.tile([128, M // 16], u16, tag="ei1")
                            e1src = (idxB_d[(t - NMAIN) * TILE_E:
                                            (t - NMAIN + 1) * TILE_E]
                                     if spill else idxAu_d[base:base + TILE_E])
                            nc.sync.dma_start(
                                out=ei1[:],
                                in_=e1src.rearrange("(p w) -> p w", p=128))
                            cand1 = cndp.tile([128, M], f32, tag="cand1")
                            nc.gpsimd.indirect_copy(
                                out=cand1[:], data=table[:], idxs=ei1[:],
                                i_know_ap_gather_is_preferred=True)
                            c1 = stp.tile([8, M], f32, tag="c1p")
                            nc.sync.dma_start(
                                out=c1[:],
                                in_=cB_d[2 * base + TILE_E:2 * base
                                         + 2 * TILE_E].rearrange(
                                    "(g m) -> g m", g=8))
                            psd1 = psdp.tile([128, M], f32, tag="psd")
                            nc.tensor.matmul(out=psd1[:], lhsT=expand8[:],
                                             rhs=c1[:], start=True, stop=True)
                            msk1 = mskp.tile([128, M], f32, tag="msk1")
                            nc.vector.scalar_tensor_tensor(
                                out=msk1[:], in0=psd1[:], scalar=iota16[:, 0:1],
                                in1=cand1[:], op0=OP.is_equal, op1=OP.mult)
                            nc.tensor.matmul(out=ps_s[:],
                                             lhsT=bdiag8[:], rhs=msk1[:],
                                             start=False, stop=True,
                                             skip_group_check=True)
                        else:
                            idx1 = idxp.tile([128, TILE_E // 16], i16, tag="idx1")
                            for G in range(8):
                                nc.sync.dma_start(
                                    out=idx1[16 * G:16 * G + 16, :],
                                    in_=idxAi_d[base:base + TILE_E].rearrange(
                                        "(c q) -> c q", c=16))
                            c1 = stp.tile([8, M], f32, tag="c1s")
                            nc.sync.dma_start(
                                out=c1[:],
                                in_=cB_d[2 * base + TILE_E:2 * base
                                         + 2 * TILE_E].rearrange(
                                    "(g m) -> g m", g=8))
                            for q in range(TILE_E // NI):
                                gth = gthp.tile([128, NI], bf16,
                                                tag=f"gth{q % 3}")
                                nc.gpsimd.dma_gather(
                                    out_ap=gth[:].rearrange(
                                        "p (one n) -> p one n", one=1),
                                    in_ap=tab16, idxs_ap=idx1[
                                        :, q * (NI // 16):(q + 1) * (NI // 16)],
                                    num_idxs=NI, num_idxs_reg=NI, elem_size=128,
                                    transpose=True, queue_num=qrr % NQ)
                                qrr += 1
                                for gg in range(NI // 512):
                                    g = q * (NI // 512) + gg
                                    psb = psbp.tile([128, M], f32, tag="psb")
                                    nc.tensor.matmul(
                                        out=psb[:], lhsT=sel8[g][:],
                                        rhs=c1[:],
                                        start=True, stop=True)
                                    msk1 = mskp.tile([128, M], f32, tag="msk1s")
                                    nc.vector.scalar_tensor_tensor(
                                        out=msk1[:], in0=psb[:],
                                        scalar=iota128[:, 0:1],
                                        in1=gth[:, 512 * gg:512 * gg + 512],
                                        op0=OP.is_equal, op1=OP.mult)
                                    nc.tensor.matmul(
                                        out=ps_s[:],
                                        lhsT=oc8[g][:], rhs=msk1[:],
                                        start=False, stop=(g == 7),
                                        skip_group_check=True)

                        tmp8 = mskp.tile([8, M], f32, tag="tmp8")
                        nc.vector.tensor_copy(out=tmp8[:], in_=ps_s[:])
                        nc.sync.dma_start(out=acc[r0:r0 + 8, :], in_=tmp8[:])

                        # ---------- gate (once per super) ----------
                        last_in_super = (t == NT - 1) or (_super_of(t + 1)[1] == 0)
                        if last_in_super:
                            sbase = s * 16 * TILE_E
                            nz = gatep.tile([P, M], f32, tag="nz")
                            nc.sync.dma_start(
                                out=nz[:],
                                in_=noise_d[sbase:sbase + P * M].rearrange(
                                    "(p m) -> p m", p=P))
                            t1 = gatep.tile([P, M], f32, tag="t1")
                            nc.scalar.activation(out=t1[:], in_=nz[:],
                                                 func=ACT.Ln,
                                                 bias=b1t[0:P, 0:1],
                                                 scale=a1t[0:P, 0:1])
                            t2 = gatep.tile([P, M], f32, tag="t2")
                            nc.scalar.activation(out=t2[:], in_=nz[:],
                                                 func=ACT.Ln,
                                                 bias=b2t[0:P, 0:1],
                                                 scale=a2t[0:P, 0:1])
                            gt = gatep.tile([P, M], f32, tag="gt")
                            nc.vector.scalar_tensor_tensor(
                                out=gt[:], in0=t1[:], scalar=0.0, in1=t2[:],
                                op0=OP.add, op1=OP.subtract)
                            gt2 = gatep.tile([P, M], f32, tag="gt2")
                            nc.vector.scalar_tensor_tensor(
                                out=gt2[:], in0=gt[:], scalar=0.0,
                                in1=acc[:], op0=OP.add, op1=OP.add)
                            ot = gatep.tile([P, M], f32, tag="ot")
                            nc.scalar.activation(out=ot[:], in_=gt2[:],
                                                 func=ACT.Sigmoid)
                            nc.sync.dma_start(
                                out=out_d[sbase:sbase + P * M].rearrange(
                                    "(p m) -> p m", p=P),
                                in_=ot[:])
    nc.compile()
    return nc


def _get_nc(repeat=1):
    if repeat not in _nc_cache:
        _nc_cache[repeat] = _build(repeat)
    return _nc_cache[repeat]


def prepare_in_maps(embedding, edges, noise, W_emb, b_emb, W_edge, b_edge):
    embedding = np.asarray(embedding, dtype=np.float32)
    edges = np.asarray(edges)
    noise = np.asarray(noise, dtype=np.float32)
    W_emb = np.asarray(W_emb, dtype=np.float32)
    b_emb = np.asarray(b_emb, dtype=np.float32)
    W_edge = np.asarray(W_edge, dtype=np.float32)
    b_edge = np.float32(b_edge)

    wbar = ((W_edge[:HID] + W_edge[HID:]) * 0.5).astype(np.float32)
    wemb = np.ascontiguousarray(W_emb.reshape(2, 128, HID))
    bemb = np.ascontiguousarray(b_emb.reshape(HID, 1))
    wbarr = np.ascontiguousarray(wbar.reshape(HID, 1))
    bhalf = np.array([[b_edge * 0.5]], dtype=np.float32)
    p = np.arange(128)
    expand8 = (p[None, :] // 16 == np.arange(8)[:, None]).astype(np.float32)
    bdiag8 = (p[:, None] // 16 == np.arange(8)[None, :]).astype(np.float32)
    iota16 = (p % 16).astype(np.float32).reshape(128, 1)
    iota128 = p.astype(np.float32).reshape(128, 1)
    sel8 = np.zeros((8, 8, 128), dtype=np.float32)
    for kk in range(8):
        sel8[kk, kk, :] = 1.0
    oc8 = np.zeros((8, 128, 8), dtype=np.float32)
    for kk in range(8):
        oc8[kk, :, kk] = 1.0

    in_maps = []
    unperm = []
    for k in range(NCORES):
        v0 = edges[0, k * EC:(k + 1) * EC].astype(np.int64)
        v1 = edges[1, k * EC:(k + 1) * EC].astype(np.int64)
        nz = noise[k * EC:(k + 1) * EC]

        b = (v0 % TAB_W).astype(np.int32)
        c0 = (v0 // TAB_W).astype(np.int32)
        ordb = np.argsort(b, kind="stable")
        bs = b[ordb]
        starts = np.searchsorted(bs, np.arange(TAB_W))
        rank = np.arange(EC) - starts[bs]
        mainm = rank < 64
        emain = ordb[mainm]                     # edge ids in main slots
        rmain = rank[mainm]
        bmain = bs[mainm]
        espill = ordb[~mainm]
        nspill = len(espill)
        assert nspill <= NSPILL * TILE_E, f"spill {nspill} overflow"

        tmain = bmain >> 6
        mmain = (bmain & 63) * 8 + (rmain & 7)
        gmain = rmain >> 3
        jmain = gmain * M + mmain               # in-tile slot id

        # spill slots (sequential fill of spill tiles)
        jsp = np.arange(nspill)
        tsp = NMAIN + jsp // TILE_E
        jsp = jsp % TILE_E
        gsp = jsp // M
        msp = jsp % M

        # device arrays
        idxAu = np.zeros(NT * TILE_E, np.uint16)
        idxAi = np.zeros(NT * TILE_E, np.int16)
        cB = np.full(NT * 2 * TILE_E, -1.0, np.float32)
        noise_dev = np.full(ECP, 0.5, np.float32)

        def wpos(t, g, m):
            # pre-wrapped u16 idx layout: partition-major contiguous
            return t * TILE_E + (g * 16 + (m % 16)) * 32 + (m // 16)

        def spos(t, j):
            # pre-wrapped i16 idx layout for dma_gather: c-major contiguous
            return t * TILE_E + (j % 16) * 256 + (j // 16)

        def nzpos(t, g, m):
            s = np.minimum(t // 16, 3)
            tt = t - 16 * s
            return s * 16 * TILE_E + (tt * 8 + g) * M + m

        # --- main slots ---
        cB[2 * tmain * TILE_E + jmain] = c0[emain]
        route_sw_t = np.array(ROUTE_SW, bool)
        is_sw = route_sw_t[tmain]
        w1 = (v1 % TAB_W).astype(np.uint16)
        c1 = (v1 // TAB_W).astype(np.float32)
        r1 = (v1 >> 7).astype(np.int16)
        cc1 = (v1 & 127).astype(np.float32)
        # POOL-routed mains
        mp = ~is_sw
        idxAu[wpos(tmain[mp], gmain[mp], mmain[mp])] = w1[emain[mp]]
        cB[(2 * tmain[mp] + 1) * TILE_E + jmain[mp]] = c1[emain[mp]]
        # SW-routed mains
        ms = is_sw
        idxAi[spos(tmain[ms], jmain[ms])] = r1[emain[ms]]
        cB[(2 * tmain[ms] + 1) * TILE_E + jmain[ms]] = cc1[emain[ms]]
        noise_dev[nzpos(tmain, gmain, mmain)] = nz[emain]

        # --- spill slots ---
        w0sp = (v0[espill] % TAB_W).astype(np.uint16)
        c0sp = (v0[espill] // TAB_W).astype(np.float32)
        idxAu[wpos(tsp, gsp, msp)] = w0sp
        cB[2 * tsp * TILE_E + jsp] = c0sp
        idxB = np.zeros(NSPILL * TILE_E, np.uint16)
        if SPILL_DST_SW:
            idxAi[spos(tsp, jsp)] = r1[espill]
            cB[(2 * tsp + 1) * TILE_E + jsp] = cc1[espill]
        else:
            idxB[wpos(tsp - NMAIN, gsp, msp)] = w1[espill]
            cB[(2 * tsp + 1) * TILE_E + jsp] = c1[espill]
        noise_dev[nzpos(tsp, gsp, msp)] = nz[espill]

        # host inverse permutation: edge -> out position
        pos = np.empty(EC, np.int64)
        pos[emain] = nzpos(tmain, gmain, mmain)
        pos[espill] = nzpos(tsp, gsp, msp)
        unperm.append(pos)

        sl = embedding[k * NNC:(k + 1) * NNC]
        embT = np.zeros((IN_DIM, NNCP), dtype=np.float32)
        embT[:, :NNC] = sl.T
        in_maps.append({
            "embT": np.ascontiguousarray(embT.reshape(2, 128, NNCP)),
            "idxAu": idxAu, "idxAi": idxAi, "idxB": idxB, "cB": cB,
            "noise": noise_dev,
            "wemb": wemb, "bemb": bemb, "wbar": wbarr, "bhalf": bhalf,
            "expand8": expand8, "bdiag8": bdiag8, "iota16": iota16,
            "iota128": iota128, "sel8": sel8, "oc8": oc8,
        })
    return in_maps, unperm


def kernel(embedding, edges, noise, W_emb, b_emb, W_edge, b_edge):
    from concourse import bass_utils
    nc = _get_nc()
    in_maps, unperm = prepare_in_maps(embedding, edges, noise, W_emb,
                                      b_emb, W_edge, b_edge)
    res = bass_utils.run_bass_kernel_spmd(nc, in_maps,
                                          core_ids=list(range(NCORES)))
    out = np.empty(E, dtype=np.float32)
    for k in range(NCORES):
        out[k * EC:(k + 1) * EC] = res.results[k]["out"][unperm[k]]
    return out
